# revision 1
# baseline (speedup 1.0000x reference)
"""Adaptive embedding (nn_AdaptiveEmbedding) Trainium2 Bass kernel.

Strategy: token-shard across 8 NeuronCores. Host routes each token to one of
6 vocab units (clusters 1 and 2 are split in half so local row indices fit
int16 for dma_gather), pads each unit's token list so every core gets an
identical count, and stages:
  - bf16 tables (narrow tables padded to 128 cols so gather rows are >=256B)
  - bf16 pre-transposed projections (scaled by sqrt(D_PROJ)), chunk-major
  - per-core int16 gather index tensors (16-partition wrapped, replicated);
    pad slots hold -1 so the gather ucode skips their descriptors
Device (per core), raw bass with explicit semaphores:
  Pool : load gather ucode library, then 6 dma_gathers (transposed layout)
  SP   : projection chunk loads
  ACT  : index load first, then per-block output stores
  PE   : per 128-token block, contract d against projection chunks in PSUM
  DVE  : PSUM -> SBUF copies (cast bf16), one per 512-col half
Host scatters the per-core output rows back to original token positions.
"""

import math
from contextlib import ExitStack

import numpy as np
import ml_dtypes

import concourse.bacc as bacc
import concourse.mybir as mybir
from concourse.bass_utils import run_bass_kernel_spmd
from concourse.library_config import mlp as _mlp_lib

N_CORES = 8
D_PROJ = 1024
EMB_SCALE = float(D_PROJ) ** 0.5

# clusters: (token_left, token_right, table_name, d, proj_idx, needs_split)
CLUSTERS = [
    (0, 20000, "emb0", 1024, 0, False),
    (20000, 60000, "emb1", 256, 1, True),   # 40000 rows > int16 range
    (60000, 100000, "emb2", 64, 2, True),   # 40000 rows > int16 range
    (100000, 128000, "emb3", 16, 3, False),
]
PJ_DIMS = [1024, 256, 64, 16]
INT16_ROWS = 32768


def _make_units(flat):
    """Units: (l, r, tname, rlo, rhi, d, pj). Split wide clusters at a
    value-aware boundary so per-core counts land near full 128-blocks
    (count_a <= 2048 -> two exactly-full blocks per core when possible).
    Order: a small unit first (quick PE start), big unit second."""
    units = []
    for (l, r, tname, d, pj, needs_split) in CLUSTERS:
        if not needs_split:
            units.append((l, r, tname, 0, r - l, d, pj))
            continue
        vals = np.sort(flat[(flat >= l) & (flat < r)])
        lo = max(l + 1, r - INT16_ROWS)
        hi = min(l + INT16_ROWS, r)
        b = int(vals[2048]) if len(vals) > 2048 else hi
        b = max(lo, min(b, hi))
        units.append((l, b, tname, 0, b - l, d, pj))
        units.append((b, r, tname, b - l, r - l, d, pj))
    # reorder: [c1b(small), c0, c1a, rest...]
    if len(units) >= 3:
        units = [units[2], units[0], units[1]] + units[3:]
    return units

BF16 = ml_dtypes.bfloat16
NPS = 4  # rotating PSUM tiles ([128,1024] f32 = 2 banks each)

# Module-level handle for test harness inspection (exec_time_ns etc).
LAST_RESULT = None


def _elem_size(d):
    """Gathered row length in bf16 elements (>=256B granularity)."""
    return max(d, 128)


def _route(flat, units):
    """Token routing: per unit, per-core counts (equal across cores),
    gather caps, token positions and local table indices."""
    rt = []
    for (l, r, tname, rlo, rhi, d, pj) in units:
        sel = (flat >= l) & (flat < r)
        pos = np.nonzero(sel)[0]
        loc = (flat[pos] - l).astype(np.int64)
        n = int(math.ceil(len(pos) / N_CORES)) if len(pos) else 0
        pad = n * N_CORES - len(pos)
        pos_p = np.concatenate([pos, np.full(pad, -1, np.int64)])
        loc_p = np.concatenate([loc, np.zeros(pad, np.int64)])
        rt.append({
            "n": n,
            "cap": ((n + 127) // 128) * 128,
            "nblocks": (n + 127) // 128,
            "pos": pos_p.reshape(N_CORES, n) if n else None,
            "loc": loc_p.reshape(N_CORES, n) if n else None,
            "d": d, "pj": pj,
        })
    return rt


def _build(units, rt):
    nc = bacc.Bacc("TRN2", target_bir_lowering=False, num_devices=N_CORES)
    act = [u for u in range(len(units)) if rt[u]["n"] > 0]

    tab_dram = {u: nc.dram_tensor(
        f"t{u}", [units[u][4] - units[u][3], _elem_size(rt[u]["d"])],
        mybir.dt.bfloat16, kind="ExternalInput") for u in act}
    idx_cols = sum(rt[u]["cap"] // 16 for u in act)
    idx_all = nc.dram_tensor("idx", [128, idx_cols], mybir.dt.int16,
                             kind="ExternalInput")
    used_pj = sorted({rt[u]["pj"] for u in act})
    pjts = {pj: nc.dram_tensor(f"p{pj}",
                               [min(PJ_DIMS[pj], 128),
                                max(PJ_DIMS[pj] // 128, 1) * D_PROJ],
                               mybir.dt.bfloat16, kind="ExternalInput")
            for pj in used_pj}
    out = nc.dram_tensor("out", [sum(rt[u]["cap"] for u in act), D_PROJ],
                         mybir.dt.bfloat16, kind="ExternalOutput")

    stack = ExitStack()
    sb = lambda name, shape, dt: stack.enter_context(
        nc.sbuf_tensor(name, list(shape), dt))
    pt_ = lambda name, shape, dt: stack.enter_context(
        nc.psum_tensor(name, list(shape), dt))
    sem = lambda name: stack.enter_context(nc.semaphore(name))

    with stack:
        it_all = sb("idxs", [128, idx_cols], mybir.dt.int16)
        idx_off, o = {}, 0
        for u in act:
            idx_off[u] = o
            o += rt[u]["cap"] // 16
        et_t = {u: sb(f"et{u}",
                      [128, _elem_size(rt[u]["d"]) // 128, rt[u]["cap"]],
                      mybir.dt.bfloat16) for u in act}
        pjt_t = {}
        n_pj_dma = 0
        for pj in used_pj:
            d = PJ_DIMS[pj]
            part, nchunk = min(d, 128), max(d // 128, 1)
            tiles = []
            for c0 in range(0, nchunk, 2):
                w = min(2, nchunk - c0)
                t = sb(f"pjt{pj}_{c0}", [part, w * D_PROJ],
                       mybir.dt.bfloat16)
                n_pj_dma += 1
                for i in range(w):
                    tiles.append((t, i, n_pj_dma))
            pjt_t[pj] = tiles
        og_t = {u: sb(f"og{u}", [128, rt[u]["nblocks"] * D_PROJ],
                      mybir.dt.bfloat16) for u in act}
        ps_t = [pt_(f"ps{i}", [128, D_PROJ], mybir.dt.float32)
                for i in range(NPS)]

        isem = sem("isem")
        psem = sem("psem")
        gsem = {u: sem(f"gsem{u}") for u in act}
        mm_sem = sem("mm_sem")
        cp_sem = sem("cp_sem")    # DVE casts
        acp_sem = sem("acp_sem")  # ACT casts
        osem = sem("osem")

        blocks = []  # (unit, b, valid-rows)
        for u in act:
            for b in range(rt[u]["nblocks"]):
                mm = min(128, rt[u]["n"] - b * 128)
                blocks.append((u, b, mm))
        # keep the first unit's blocks first (earliest gather -> quick PE
        # start), then full blocks, then ragged tails (small-m casts drain
        # the PSUM->SBUF pipeline fastest at the very end)
        first_u = blocks[0][0]
        blocks = ([bl for bl in blocks if bl[0] == first_u] +
                  [bl for bl in blocks if bl[0] != first_u and bl[2] == 128] +
                  sorted([bl for bl in blocks
                          if bl[0] != first_u and bl[2] < 128],
                         key=lambda bl: -bl[2]))
        out_off, oo = {}, 0
        for u in act:
            out_off[u] = oo
            oo += rt[u]["cap"]

        if True:

            def _(gp):
                gp.load_library(_mlp_lib)
                # one register per distinct valid-count (fewer serial MOVEs)
                regs = {}
                for u in act:
                    n = rt[u]["n"]
                    if n not in regs:
                        regs[n] = gp.to_reg(n)
                gp.wait_ge(isem, 16)
                for u in act:
                    gp.dma_gather(
                        et_t[u][:], tab_dram[u][:],
                        it_all[:, idx_off[u]: idx_off[u] + rt[u]["cap"] // 16],
                        rt[u]["cap"], regs[rt[u]["n"]],
                        _elem_size(rt[u]["d"]),
                        transpose=True,
                    ).then_inc(gsem[u], 16)

            # halves 2i+h; the last blocks' h1 casts run on ACT so the DVE
            # queue doesn't trail the final matmuls. rank maps give each
            # cast a deterministic per-engine completion count.
            L = len(blocks)
            act_half = set()
            dve_rank, act_rank = {}, {}
            for j in range(2 * L):
                if j in act_half:
                    act_rank[j] = len(act_rank) + 1
                else:
                    dve_rank[j] = len(dve_rank) + 1

            _(nc.gpsimd)

            def _(sc):
                sc.dma_start(it_all[:], idx_all[:]).then_inc(isem, 16)
                for i, (u, b, mm) in enumerate(blocks):
                    if 2 * i + 1 in act_half:
                        sc.wait_ge(mm_sem, 2 * i + 2)
                        sc.copy(
                            og_t[u][:mm, b * D_PROJ + 512:
                                    b * D_PROJ + 1024],
                            ps_t[i % NPS][:mm, 512:1024],
                        ).then_inc(acp_sem, 1)
                    # both halves of this block copied -> store it
                    need_dve = max((dve_rank[j] for j in (2 * i, 2 * i + 1)
                                    if j in dve_rank), default=0)
                    if need_dve:
                        sc.wait_ge(cp_sem, need_dve)
                    r0 = out_off[u] + b * 128
                    sc.dma_start(
                        out[r0:r0 + 128, :],
                        og_t[u][:, b * D_PROJ:(b + 1) * D_PROJ],
                    ).then_inc(osem, 16)
                sc.wait_ge(osem, 16 * len(blocks))

            _(nc.scalar)

            def _(sy):
                for pj in used_pj:
                    d = PJ_DIMS[pj]
                    nchunk = max(d // 128, 1)
                    for c0 in range(0, nchunk, 2):
                        w = min(2, nchunk - c0)
                        t = pjt_t[pj][c0][0]
                        sy.dma_start(
                            t[:], pjts[pj][:, c0 * D_PROJ:(c0 + w) * D_PROJ]
                        ).then_inc(psem, 16)

            _(nc.sync)

            def _(te):
                seen = set()
                for i, (u, b, mm) in enumerate(blocks):
                    pj = rt[u]["pj"]
                    kdim = min(rt[u]["d"], 128)
                    nchunk = max(rt[u]["d"] // 128, 1)
                    if u not in seen:
                        seen.add(u)
                        te.wait_ge(gsem[u], 16)
                        need = max(dma_i for _, _, dma_i in pjt_t[pj])
                        te.wait_ge(psem, 16 * need)
                    if i >= NPS:
                        tgt = 2 * (i - NPS) + 1
                        if tgt in dve_rank:
                            te.wait_ge(cp_sem, dve_rank[tgt])
                        else:
                            te.wait_ge(acp_sem, act_rank[tgt])
                    ps = ps_t[i % NPS]
                    for h in range(2):
                        last = None
                        for c in range(nchunk):
                            pt, ci, _ = pjt_t[pj][c]
                            last = te.matmul(
                                ps[:mm, h * 512:(h + 1) * 512],
                                et_t[u][:kdim, c, b * 128: b * 128 + mm],
                                pt[:kdim, ci * D_PROJ + h * 512:
                                   ci * D_PROJ + h * 512 + 512],
                                start=(c == 0),
                                stop=(c == nchunk - 1),
                            )
                        last.then_inc(mm_sem, 1)

            _(nc.tensor)

            def _(ve):
                for i, (u, b, mm) in enumerate(blocks):
                    for h in range(2):
                        if 2 * i + h in act_half:
                            continue
                        ve.wait_ge(mm_sem, 2 * i + h + 1)
                        ve.tensor_copy(
                            og_t[u][:mm, b * D_PROJ + h * 512:
                                    b * D_PROJ + (h + 1) * 512],
                            ps_t[i % NPS][:mm, h * 512:(h + 1) * 512],
                        ).then_inc(cp_sem, 1)

            _(nc.vector)

        nc.compile()
    return nc


def kernel(input, emb0, emb1, emb2, emb3, proj0, proj1, proj2, proj3):
    global LAST_RESULT
    inp = np.asarray(input)
    flat = inp.reshape(-1).astype(np.int64)
    T = flat.shape[0]
    tables = {"emb0": np.asarray(emb0), "emb1": np.asarray(emb1),
              "emb2": np.asarray(emb2), "emb3": np.asarray(emb3)}
    projs = [np.asarray(proj0), np.asarray(proj1),
             np.asarray(proj2), np.asarray(proj3)]

    units = _make_units(flat)
    rt = _route(flat, units)
    act = [u for u in range(len(units)) if rt[u]["n"] > 0]

    # --- stage tables (bf16, narrow rows padded to 128 cols) ---------------
    tab_stage = {}
    for u in act:
        l, r, tname, rlo, rhi, d, pj = units[u]
        sl = tables[tname][rlo:rhi].astype(BF16)
        es = _elem_size(d)
        if es != d:
            padded = np.zeros((sl.shape[0], es), dtype=BF16)
            padded[:, :d] = sl
            sl = padded
        tab_stage[u] = np.ascontiguousarray(sl)

    # --- stage projections: (proj.T * EMB_SCALE), chunk-major bf16 ---------
    pjt_stage = [None] * 4
    for pj in range(4):
        d = projs[pj].shape[1]
        pt = (projs[pj].T.astype(np.float32) * EMB_SCALE)  # [d, D_PROJ]
        if d >= 128:
            nchunk = d // 128
            pt = pt.reshape(nchunk, 128, D_PROJ).transpose(1, 0, 2)
            pt = pt.reshape(128, nchunk * D_PROJ)
        pjt_stage[pj] = np.ascontiguousarray(pt.astype(BF16))

    # --- per-core combined index tensor (pads = -1) ------------------------
    idx_stage = []
    for k in range(N_CORES):
        parts = []
        for u in act:
            cap = rt[u]["cap"]
            full = np.full(cap, -1, np.int16)
            full[:rt[u]["n"]] = rt[u]["loc"][k].astype(np.int16)
            parts.append(np.tile(full.reshape(cap // 16, 16).T, (8, 1)))
        idx_stage.append(np.ascontiguousarray(np.concatenate(parts, axis=1)))

    # --- build + run -------------------------------------------------------
    nc = _build(units, rt)
    in_maps = []
    for k in range(N_CORES):
        mm = {"idx": idx_stage[k]}
        for u in act:
            mm[f"t{u}"] = tab_stage[u]
        for pj in sorted({rt[u]["pj"] for u in act}):
            mm[f"p{pj}"] = pjt_stage[pj]
        in_maps.append(mm)

    res = run_bass_kernel_spmd(nc, in_maps, core_ids=list(range(N_CORES)))
    LAST_RESULT = res

    # --- unpermute ---------------------------------------------------------
    out_full = np.zeros((T, D_PROJ), np.float32)
    for k in range(N_CORES):
        rows = res.results[k]["out"]
        off = 0
        for u in act:
            pos = rt[u]["pos"][k]
            valid = pos >= 0
            seg = rows[off: off + rt[u]["n"]]
            out_full[pos[valid]] = seg[valid].astype(np.float32)
            off += rt[u]["cap"]
    return out_full.reshape(*inp.shape, D_PROJ)



# revision 16
# speedup vs baseline: 1.0111x; 1.0111x over previous
"""Adaptive embedding (nn_AdaptiveEmbedding) Trainium2 Bass kernel, v2.

Design: one SPMD program with per-core specialized sections dispatched via a
partition_id() If-tree.  Host routes tokens to cores so each core serves a
small set of clusters (cluster-specialized sharding) -- this removes the 8x
replication of the projection matrices that dominated HBM traffic in v1.

Per 128-token block, the device:
  Pool : one indirect_dma_start (HW dynamic-offset DMA, no ucode library)
         gathering 128 table rows -> SBUF [tokens, d] (token per partition)
  PE   : transposes the gathered tile chunk-wise to [d, tokens] (via identity
         matmul into bf16 PSUM), then runs the projection matmuls into f32
         PSUM, software-pipelined one block ahead of the transposes
  DVE  : copies transposed chunks PSUM->SBUF and casts output half 0
  ACT  : casts output half 1
  SP   : loads idx/identity/projection tiles, stores finished blocks

Host scatters per-core block outputs back to original token positions.
"""

import math
from contextlib import ExitStack

import numpy as np
import ml_dtypes

import concourse.bacc as bacc
import concourse.bass as bass
import concourse.mybir as mybir
from concourse.bass_utils import run_bass_kernel_spmd

N_CORES = 8
D_PROJ = 1024
EMB_SCALE = float(D_PROJ) ** 0.5
BF16 = ml_dtypes.bfloat16

# clusters: (token_left, token_right, d)
CLUSTERS = [
    (0, 20000, 1024),
    (20000, 60000, 256),
    (60000, 100000, 64),
    (100000, 128000, 16),
]

# cost model for the balancer (ns)
POOL_PER_BLOCK = 1250.0
PE_NS_PER_COL = 0.5          # ~2 GHz effective (p-state mix)
DMA_NS_PER_BYTE = 1.0 / 300.0e9 * 1e9   # 300 GB/s
POOL_START, PE_START, DMA_START = 9500.0, 12000.0, 8300.0
# max number of cores each cluster's blocks (and proj copy) may spread to
SPREAD_CAP = [4, 3, 4, 6]

NB_MAX = 16          # max blocks per core the program supports
G_COLS = 8192        # gather buffer cols (bf16) per partition
NPS = 2              # psum double-buffer depth (out tiles and T tiles)
NO_DISPATCH = False  # debug: emit plans[0] for every core, no branching

LAST_RESULT = None


def _pe_cols(d):
    nch = max(d // 128, 1)
    kd = min(d, 128)
    return nch * 128 + nch * 2 * 512  # transposes + matmuls (free-dim cols)


def _block_bytes(d):
    return 128 * d * 2 + 128 * D_PROJ * 2  # gather + out (bf16)


PJ_BYTES = [2 * 1024 * 1024, 512 * 1024, 128 * 1024, 32 * 1024]


def _route(flat):
    """Token routing per cluster: sorted positions and local indices."""
    out = []
    for (l, r, d) in CLUSTERS:
        sel = (flat >= l) & (flat < r)
        pos = np.nonzero(sel)[0]
        loc = (flat[pos] - l).astype(np.int64)
        order = np.argsort(loc, kind="stable")
        out.append({"pos": pos[order], "loc": loc[order], "d": d, "n": len(pos)})
    return out


def _balance(rt):
    """Block assignment minimizing max per-core makespan, with a hard cap on
    how many cores each cluster (and its proj copy) may spread to.

    Returns cores: list of 8 dicts with
      blocks: list of (cluster, start, end) token ranges (<=128 each)
      pj: set of cluster ids present
    Token ranges index into the cluster's sorted token arrays, so each
    core gets a contiguous slice of the sorted-by-loc token list (compact
    vocab slice per core).
    """
    cores = [{"blocks": [], "pj": set(), "pool": 0.0, "pe": 0.0, "dma": 0.0}
             for _ in range(N_CORES)]

    def span(c, dpool=0.0, dpe=0.0, ddma=0.0, pj_extra=0):
        pjb = sum(PJ_BYTES[p] for p in c["pj"]) + pj_extra
        return max(POOL_START + c["pool"] + dpool,
                   PE_START + c["pe"] + dpe,
                   DMA_START + (c["dma"] + ddma + pjb) * DMA_NS_PER_BYTE)

    for ci in [0, 1, 2, 3]:
        n = rt[ci]["n"]
        nblk = (n + 127) // 128
        d = rt[ci]["d"]
        pe_b = _pe_cols(d) * PE_NS_PER_COL
        by_b = _block_bytes(d)
        counts = [0] * N_CORES
        for _ in range(nblk):
            # candidate cores: those already serving ci, or (if spread cap
            # not hit) any core
            have = [k for k in range(N_CORES) if ci in cores[k]["pj"]]
            cands = list(range(N_CORES)) if len(have) < SPREAD_CAP[ci] else have
            best, bestv = None, None
            for k in cands:
                c = cores[k]
                extra_pj = 0 if ci in c["pj"] else PJ_BYTES[ci]
                v = span(c, POOL_PER_BLOCK, pe_b, by_b, extra_pj)
                # tie-break: prefer cores that already carry this proj
                v += (0 if ci in c["pj"] else 1.0)
                if bestv is None or v < bestv - 1e-9:
                    bestv, best = v, k
            c = cores[best]
            c["pool"] += POOL_PER_BLOCK
            c["pe"] += pe_b
            c["dma"] += by_b
            c["pj"].add(ci)
            counts[best] += 1
        start = 0
        for k in range(N_CORES):
            for _ in range(counts[k]):
                end = min(start + 128, n)
                cores[k]["blocks"].append((ci, start, end))
                start = end
        assert start == n
    return cores


def _build(plans):
    """plans[k]: list of block descriptors:
       (cluster, d, tab_row_offset_base, nblk_index, mm) plus idx data handled
       by host.  We need per-core: blocks list with (cluster, mm)."""
    nc = bacc.Bacc("TRN2", target_bir_lowering=False, num_devices=N_CORES)

    # table shapes: max rows over cores per cluster (host pads)
    tab_rows = [max((p["tab_rows"][ci] for p in plans), default=1) or 1
                for ci in range(4)]
    tabs = [nc.dram_tensor(f"tab{ci}", [max(tab_rows[ci], 1), CLUSTERS[ci][2]],
                           mybir.dt.bfloat16, kind="ExternalInput")
            for ci in range(4)]
    idxd = nc.dram_tensor("idxd", [128, NB_MAX], mybir.dt.int32,
                          kind="ExternalInput")
    iden = nc.dram_tensor("iden", [128, 128], mybir.dt.bfloat16,
                          kind="ExternalInput")
    pjts = [nc.dram_tensor(f"pjt{ci}", [min(CLUSTERS[ci][2], 128),
                                        max(CLUSTERS[ci][2] // 128, 1) * D_PROJ],
                           mybir.dt.bfloat16, kind="ExternalInput")
            for ci in range(4)]
    outD = nc.dram_tensor("out", [NB_MAX * 128, D_PROJ], mybir.dt.bfloat16,
                          kind="ExternalOutput")

    stack = ExitStack()
    sb = lambda name, shape, dt: stack.enter_context(
        nc.sbuf_tensor(name, list(shape), dt))
    pt_ = lambda name, shape, dt: stack.enter_context(
        nc.psum_tensor(name, list(shape), dt))
    sem = lambda name: stack.enter_context(nc.semaphore(name))

    with stack:
        idxt = sb("idxt", [128, NB_MAX], mybir.dt.int32)
        idn = sb("idn", [128, 128], mybir.dt.bfloat16)
        G = sb("G", [128, G_COLS], mybir.dt.bfloat16)
        ET = [sb(f"ET{i}", [128, 1024], mybir.dt.bfloat16) for i in range(NPS)]
        OG = sb("OG", [128, NB_MAX * D_PROJ], mybir.dt.bfloat16)
        pjt_sb = [sb(f"pj{ci}", [min(CLUSTERS[ci][2], 128),
                                 max(CLUSTERS[ci][2] // 128, 1) * D_PROJ],
                     mybir.dt.bfloat16) for ci in range(4)]
        # transposes must land at a PSUM bank base: ping-pong two bank-sized
        # slots, each transpose writes cols 0:128 of its slot
        psT = [pt_(f"psT{i}", [128, 1024], mybir.dt.bfloat16)
               for i in range(2)]
        psO = [pt_(f"psO{i}", [128, D_PROJ], mybir.dt.float32)
               for i in range(NPS)]

        isem = sem("isem")    # idx load
        idsem = sem("idsem")  # identity load
        psem = [sem(f"psem{i}") for i in range(7)]   # proj tile loads
        gsem = [sem(f"gsem{i}") for i in range(NB_MAX)]  # per-block gathers
        tsem = sem("tsem")    # PE transposes
        csem = sem("csem")    # DVE chunk copies
        mmsem = sem("mmsem")  # matmul halves
        vcsem = sem("vcsem")  # DVE out casts (h0)
        acsem = sem("acsem")  # ACT out casts (h1)
        osem = sem("osem")    # stores

        # per-core proj DMA schedule: list of (cluster, chunk_lo, n_chunks)
        # c0 is split into 4 DMAs of 2 chunks; others one DMA each.
        def proj_dmas(pjset):
            sched = []
            for ci in sorted(pjset):
                nch = max(CLUSTERS[ci][2] // 128, 1)
                if ci == 0:
                    for c0 in range(0, nch, 2):
                        sched.append((ci, c0, 2))
                else:
                    sched.append((ci, 0, nch))
            return sched

        def section_sync(sy, plan):
            sy.dma_start(idxt[:, :], idxd[:, :]).then_inc(isem, 16)
            sy.dma_start(idn[:, :], iden[:, :]).then_inc(idsem, 16)
            for i, (ci, c0, w) in enumerate(proj_dmas(plan["pj"])):
                part = min(CLUSTERS[ci][2], 128)
                sy.dma_start(
                    pjt_sb[ci][:part, c0 * D_PROJ:(c0 + w) * D_PROJ],
                    pjts[ci][:part, c0 * D_PROJ:(c0 + w) * D_PROJ],
                ).then_inc(psem[i], 16)
            # stores
            nvc = nac = 0
            for b, blk in enumerate(plan["blocks"]):
                nvc += 1
                nac += 1
                sy.wait_ge(vcsem, nvc)
                sy.wait_ge(acsem, nac)
                sy.dma_start(
                    outD[b * 128: b * 128 + blk["mm"], :],
                    OG[:blk["mm"], b * D_PROJ:(b + 1) * D_PROJ],
                ).then_inc(osem, 16)
            sy.wait_ge(osem, 16 * len(plan["blocks"]))

        def section_pool(gp, plan):
            gp.wait_ge(isem, 16)
            for b, blk in enumerate(plan["blocks"]):
                ci = blk["ci"]
                d = CLUSTERS[ci][2]
                gp.indirect_dma_start(
                    G[:, blk["goff"]: blk["goff"] + d], None,
                    tabs[ci][:, :],
                    bass.IndirectOffsetOnAxis(ap=idxt[:, b:b + 1], axis=0),
                ).then_inc(gsem[b], 16)

        def section_pe(te, plan):
            blocks = plan["blocks"]
            pj_sched = proj_dmas(plan["pj"])
            # dma index (0-based) needed for cluster ci chunk c
            def pj_need(ci, c):
                for i, (cj, c0, w) in enumerate(pj_sched):
                    if cj == ci and c0 <= c < c0 + w:
                        return i
                raise AssertionError

            te.wait_ge(idsem, 16)

            cum_copies = [0] * (len(blocks) + 1)
            for b, blk in enumerate(blocks):
                d = CLUSTERS[blk["ci"]][2]
                cum_copies[b + 1] = cum_copies[b] + max(d // 128, 1)

            def emit_T(b):
                blk = blocks[b]
                d = CLUSTERS[blk["ci"]][2]
                nch = max(d // 128, 1)
                te.wait_ge(gsem[b], 16)
                for c in range(nch):
                    w = min(128, d - c * 128)
                    t = cum_copies[b] + c
                    if t >= 2:
                        te.wait_ge(csem, t - 1)
                    te.transpose(
                        psT[t % 2][:w, 0:128],
                        G[:, blk["goff"] + c * 128: blk["goff"] + c * 128 + w],
                        idn[:, :],
                    ).then_inc(tsem, 1)

            def emit_MM(b, copies_done):
                blk = blocks[b]
                ci = blk["ci"]
                d = CLUSTERS[ci][2]
                nch = max(d // 128, 1)
                kd = min(d, 128)
                mm = blk["mm"]
                te.wait_ge(csem, copies_done)
                if b >= NPS:
                    # psO reuse: casts of block b-NPS must be done
                    te.wait_ge(vcsem, b - NPS + 1)
                    te.wait_ge(acsem, b - NPS + 1)
                seen = plan.setdefault("_pj_seen", set())
                for h in range(2):
                    last = None
                    for c in range(nch):
                        i_pj = pj_need(ci, c)
                        if (ci, i_pj) not in seen:
                            te.wait_ge(psem[i_pj], 16)
                            seen.add((ci, i_pj))
                        last = te.matmul(
                            psO[b % NPS][:mm, h * 512:(h + 1) * 512],
                            ET[b % NPS][:kd, c * 128: c * 128 + mm],
                            pjt_sb[ci][:kd, c * D_PROJ + h * 512:
                                       c * D_PROJ + h * 512 + 512],
                            start=(c == 0),
                            stop=(c == nch - 1),
                        )
                    last.then_inc(mmsem, 1)

            # software pipeline: T(0); for b: T(b+1); MM(b)
            emit_T(0)
            for b in range(len(blocks)):
                if b + 1 < len(blocks):
                    emit_T(b + 1)
                emit_MM(b, cum_copies[b + 1])

        def section_dve(ve, plan):
            blocks = plan["blocks"]
            nt = 0
            # chunk copies follow transposes in the same order PE emits them
            order = []
            order.append(("T", 0))
            for b in range(len(blocks)):
                if b + 1 < len(blocks):
                    order.append(("T", b + 1))
                order.append(("M", b))
            nmm = 0
            for kind, b in order:
                blk = blocks[b]
                d = CLUSTERS[blk["ci"]][2]
                nch = max(d // 128, 1)
                if kind == "T":
                    for c in range(nch):
                        w = min(128, d - c * 128)
                        ve.wait_ge(tsem, nt + 1)
                        ve.tensor_copy(
                            ET[b % NPS][:w, c * 128:(c + 1) * 128],
                            psT[nt % 2][:w, 0:128],
                        ).then_inc(csem, 1)
                        nt += 1
                else:
                    mm = blk["mm"]
                    nmm += 2
                    ve.wait_ge(mmsem, nmm - 1)
                    ve.tensor_copy(
                        OG[:mm, b * D_PROJ: b * D_PROJ + 512],
                        psO[b % NPS][:mm, 0:512],
                    ).then_inc(vcsem, 1)

        def section_act(sc, plan):
            blocks = plan["blocks"]
            for b, blk in enumerate(blocks):
                mm = blk["mm"]
                sc.wait_ge(mmsem, 2 * (b + 1))
                sc.copy(
                    OG[:mm, b * D_PROJ + 512: b * D_PROJ + 1024],
                    psO[b % NPS][:mm, 512:1024],
                ).then_inc(acsem, 1)

        def dispatch(eng, emit):
            if NO_DISPATCH:
                emit(eng, plans[0])
                return
            pid = eng.partition_id()
            with eng.If_lt(pid, 4):
                with eng.If_lt(pid, 2):
                    with eng.If_eq(pid, 0):
                        emit(eng, plans[0])
                    with eng.Else():
                        emit(eng, plans[1])
                with eng.Else():
                    with eng.If_eq(pid, 2):
                        emit(eng, plans[2])
                    with eng.Else():
                        emit(eng, plans[3])
            with eng.Else():
                with eng.If_lt(pid, 6):
                    with eng.If_eq(pid, 4):
                        emit(eng, plans[4])
                    with eng.Else():
                        emit(eng, plans[5])
                with eng.Else():
                    with eng.If_eq(pid, 6):
                        emit(eng, plans[6])
                    with eng.Else():
                        emit(eng, plans[7])

        def _(sy):
            dispatch(sy, section_sync)
        _(nc.sync)

        def _(gp):
            dispatch(gp, section_pool)
        _(nc.gpsimd)

        def _(te):
            def emit(eng, plan):
                plan.pop("_pj_seen", None)
                section_pe(eng, plan)
            dispatch(te, emit)
        _(nc.tensor)

        def _(ve):
            dispatch(ve, section_dve)
        _(nc.vector)

        def _(sc):
            dispatch(sc, section_act)
        _(nc.scalar)

        nc.compile()
    return nc


def kernel(input, emb0, emb1, emb2, emb3, proj0, proj1, proj2, proj3):
    global LAST_RESULT
    inp = np.asarray(input)
    flat = inp.reshape(-1).astype(np.int64)
    T = flat.shape[0]
    tables = [np.asarray(emb0), np.asarray(emb1), np.asarray(emb2),
              np.asarray(emb3)]
    projs = [np.asarray(proj0), np.asarray(proj1), np.asarray(proj2),
             np.asarray(proj3)]

    rt = _route(flat)
    cores = _balance(rt)

    # --- build per-core plans ---------------------------------------------
    plans = []
    for k in range(N_CORES):
        c = cores[k]
        blocks = []
        goff = 0
        tab_lo = {}  # cluster -> (lo_loc, hi_loc)
        for (ci, s, e) in c["blocks"]:
            loc = rt[ci]["loc"][s:e]
            lo, hi = tab_lo.get(ci, (1 << 60, -1))
            tab_lo[ci] = (min(lo, int(loc.min())), max(hi, int(loc.max())))
        plan = {"pj": c["pj"], "blocks": [], "tab_rows": [0] * 4,
                "tab_base": {}}
        for ci, (lo, hi) in tab_lo.items():
            plan["tab_base"][ci] = lo
            plan["tab_rows"][ci] = hi - lo + 1
        for (ci, s, e) in c["blocks"]:
            d = CLUSTERS[ci][2]
            plan["blocks"].append({
                "ci": ci, "s": s, "e": e, "mm": e - s, "goff": goff,
            })
            goff += d
        assert goff <= G_COLS, f"core {k}: G overflow {goff}"
        assert len(plan["blocks"]) <= NB_MAX
        plans.append(plan)

    nc = _build(plans)

    # --- stage host data ---------------------------------------------------
    tab_rows_max = [max(max((p["tab_rows"][ci] for p in plans)), 1)
                    for ci in range(4)]
    pjt_stage = []
    for ci in range(4):
        d = CLUSTERS[ci][2]
        pt = projs[ci].T.astype(np.float32) * EMB_SCALE  # [d, D_PROJ]
        if d >= 128:
            nch = d // 128
            pt = pt.reshape(nch, 128, D_PROJ).transpose(1, 0, 2)
            pt = pt.reshape(128, nch * D_PROJ)
        pjt_stage.append(np.ascontiguousarray(pt.astype(BF16)))
    iden_np = np.eye(128, dtype=np.float32).astype(BF16)

    in_maps = []
    for k in range(N_CORES):
        plan = plans[k]
        mm = {"iden": iden_np}
        for ci in range(4):
            rows = tab_rows_max[ci]
            d = CLUSTERS[ci][2]
            arr = np.zeros((rows, d), dtype=BF16)
            if plan["tab_rows"][ci] > 0:
                base = plan["tab_base"][ci]
                n = plan["tab_rows"][ci]
                arr[:n] = tables[ci][base: base + n].astype(BF16)
            mm[f"tab{ci}"] = arr
            mm[f"pjt{ci}"] = pjt_stage[ci]
        idx = np.zeros((128, NB_MAX), dtype=np.int32)
        for b, blk in enumerate(plan["blocks"]):
            ci = blk["ci"]
            loc = rt[ci]["loc"][blk["s"]: blk["e"]] - plan["tab_base"][ci]
            idx[: blk["mm"], b] = loc.astype(np.int32)
        mm["idxd"] = idx
        in_maps.append(mm)

    res = run_bass_kernel_spmd(nc, in_maps, core_ids=list(range(N_CORES)))
    LAST_RESULT = res

    # --- unpermute ---------------------------------------------------------
    out_full = np.zeros((T, D_PROJ), np.float32)
    for k in range(N_CORES):
        rows = res.results[k]["out"]
        for b, blk in enumerate(plans[k]["blocks"]):
            ci = blk["ci"]
            pos = rt[ci]["pos"][blk["s"]: blk["e"]]
            out_full[pos] = rows[b * 128: b * 128 + blk["mm"]].astype(
                np.float32)
    return out_full.reshape(*inp.shape, D_PROJ)


# revision 23
# speedup vs baseline: 1.0428x; 1.0313x over previous
"""Adaptive embedding (nn_AdaptiveEmbedding) Trainium2 Bass kernel, v2.

Design: one SPMD program with per-core specialized sections dispatched via a
partition_id() If-tree.  Host routes tokens to cores so each core serves a
small set of clusters (cluster-specialized sharding) -- this removes the 8x
replication of the projection matrices that dominated HBM traffic in v1.

Per 128-token block, the device:
  Pool : one indirect_dma_start (HW dynamic-offset DMA, no ucode library)
         gathering 128 table rows -> SBUF [tokens, d] (token per partition)
  PE   : transposes the gathered tile chunk-wise to [d, tokens] (via identity
         matmul into bf16 PSUM), then runs the projection matmuls into f32
         PSUM, software-pipelined one block ahead of the transposes
  DVE  : copies transposed chunks PSUM->SBUF and casts output half 0
  ACT  : casts output half 1
  SP   : loads idx/identity/projection tiles, stores finished blocks

Host scatters per-core block outputs back to original token positions.
"""

import math
from contextlib import ExitStack

import numpy as np
import ml_dtypes

import concourse.bacc as bacc
import concourse.bass as bass
import concourse.mybir as mybir
from concourse.bass_utils import run_bass_kernel_spmd

N_CORES = 8
D_PROJ = 1024
EMB_SCALE = float(D_PROJ) ** 0.5
BF16 = ml_dtypes.bfloat16

# clusters: (token_left, token_right, d)
CLUSTERS = [
    (0, 20000, 1024),
    (20000, 60000, 256),
    (60000, 100000, 64),
    (100000, 128000, 16),
]

# cost model for the balancer (ns)
POOL_PER_BLOCK = 1700.0
PE_NS_PER_COL = 0.5          # ~2 GHz effective (p-state mix)
DMA_NS_PER_BYTE = 1.0 / 300.0e9 * 1e9   # 300 GB/s
POOL_START, PE_START, DMA_START = 10500.0, 12000.0, 8300.0
VEC_START = 11000.0
# max number of cores each cluster's blocks (and proj copy) may spread to
SPREAD_CAP = [4, 3, 4, 6]


def _dve_ns(d):
    return max(d // 128, 1) * 300.0 + 530.0  # T-copies + h0 cast

NB_MAX = 16          # max blocks per core the program supports
G_COLS = 8192        # gather buffer cols (bf16) per partition
NPS = 2              # psum double-buffer depth (out tiles and T tiles)
NO_DISPATCH = False  # debug: emit plans[0] for every core, no branching

LAST_RESULT = None


def _pe_cols(d):
    nch = max(d // 128, 1)
    kd = min(d, 128)
    return nch * 128 + nch * 2 * 512  # transposes + matmuls (free-dim cols)


def _block_bytes(d):
    return 128 * d * 2 + 128 * D_PROJ * 2  # gather + out (bf16)


PJ_BYTES = [2 * 1024 * 1024, 512 * 1024, 128 * 1024, 32 * 1024]


def _route(flat):
    """Token routing per cluster: sorted positions and local indices."""
    out = []
    for (l, r, d) in CLUSTERS:
        sel = (flat >= l) & (flat < r)
        pos = np.nonzero(sel)[0]
        loc = (flat[pos] - l).astype(np.int64)
        order = np.argsort(loc, kind="stable")
        out.append({"pos": pos[order], "loc": loc[order], "d": d, "n": len(pos)})
    return out


def _balance(rt):
    """Block assignment minimizing max per-core makespan, with a hard cap on
    how many cores each cluster (and its proj copy) may spread to.

    Returns cores: list of 8 dicts with
      blocks: list of (cluster, start, end) token ranges (<=128 each)
      pj: set of cluster ids present
    Token ranges index into the cluster's sorted token arrays, so each
    core gets a contiguous slice of the sorted-by-loc token list (compact
    vocab slice per core).
    """
    cores = [{"blocks": [], "pj": set(), "pool": 0.0, "pe": 0.0, "dma": 0.0,
              "dve": 0.0}
             for _ in range(N_CORES)]

    def span(c, dpool=0.0, dpe=0.0, ddma=0.0, pj_extra=0, ddve=0.0):
        pjb = sum(PJ_BYTES[p] for p in c["pj"]) + pj_extra
        return max(POOL_START + c["pool"] + dpool,
                   PE_START + c["pe"] + dpe,
                   VEC_START + c["dve"] + ddve,
                   DMA_START + (c["dma"] + ddma + pjb) * DMA_NS_PER_BYTE)

    for ci in [0, 1, 2, 3]:
        n = rt[ci]["n"]
        nblk = (n + 127) // 128
        d = rt[ci]["d"]
        pe_b = _pe_cols(d) * PE_NS_PER_COL
        by_b = _block_bytes(d)
        dve_b = _dve_ns(d)
        counts = [0] * N_CORES
        for _ in range(nblk):
            # candidate cores: those already serving ci, or (if spread cap
            # not hit) any core
            have = [k for k in range(N_CORES) if ci in cores[k]["pj"]]
            cands = list(range(N_CORES)) if len(have) < SPREAD_CAP[ci] else have
            best, bestv = None, None
            for k in cands:
                c = cores[k]
                extra_pj = 0 if ci in c["pj"] else PJ_BYTES[ci]
                v = span(c, POOL_PER_BLOCK, pe_b, by_b, extra_pj, dve_b)
                # tie-break: prefer cores that already carry this proj
                v += (0 if ci in c["pj"] else 1.0)
                if bestv is None or v < bestv - 1e-9:
                    bestv, best = v, k
            c = cores[best]
            c["pool"] += POOL_PER_BLOCK
            c["pe"] += pe_b
            c["dma"] += by_b
            c["dve"] += dve_b
            c["pj"].add(ci)
            counts[best] += 1
        start = 0
        for k in range(N_CORES):
            for _ in range(counts[k]):
                end = min(start + 128, n)
                cores[k]["blocks"].append((ci, start, end))
                start = end
        assert start == n
    return cores


def _build(plans):
    """plans[k]: list of block descriptors:
       (cluster, d, tab_row_offset_base, nblk_index, mm) plus idx data handled
       by host.  We need per-core: blocks list with (cluster, mm)."""
    nc = bacc.Bacc("TRN2", target_bir_lowering=False, num_devices=N_CORES)

    # table shapes: max rows over cores per cluster (host pads)
    tab_rows = [max((p["tab_rows"][ci] for p in plans), default=1) or 1
                for ci in range(4)]
    tabs = [nc.dram_tensor(f"tab{ci}", [max(tab_rows[ci], 1), CLUSTERS[ci][2]],
                           mybir.dt.bfloat16, kind="ExternalInput")
            for ci in range(4)]
    idxd = nc.dram_tensor("idxd", [128, NB_MAX], mybir.dt.int32,
                          kind="ExternalInput")
    iden = nc.dram_tensor("iden", [128, 128], mybir.dt.bfloat16,
                          kind="ExternalInput")
    pjts = [nc.dram_tensor(f"pjt{ci}", [min(CLUSTERS[ci][2], 128),
                                        max(CLUSTERS[ci][2] // 128, 1) * D_PROJ],
                           mybir.dt.bfloat16, kind="ExternalInput")
            for ci in range(4)]
    outD = nc.dram_tensor("out", [NB_MAX * 128, D_PROJ], mybir.dt.bfloat16,
                          kind="ExternalOutput")

    stack = ExitStack()
    sb = lambda name, shape, dt: stack.enter_context(
        nc.sbuf_tensor(name, list(shape), dt))
    pt_ = lambda name, shape, dt: stack.enter_context(
        nc.psum_tensor(name, list(shape), dt))
    sem = lambda name: stack.enter_context(nc.semaphore(name))

    with stack:
        idxt = sb("idxt", [128, NB_MAX], mybir.dt.int32)
        idn = sb("idn", [128, 128], mybir.dt.bfloat16)
        G = sb("G", [128, G_COLS], mybir.dt.bfloat16)
        ET = [sb(f"ET{i}", [128, 1024], mybir.dt.bfloat16) for i in range(NPS)]
        OG = sb("OG", [128, NB_MAX * D_PROJ], mybir.dt.bfloat16)
        pjt_sb = [sb(f"pj{ci}", [min(CLUSTERS[ci][2], 128),
                                 max(CLUSTERS[ci][2] // 128, 1) * D_PROJ],
                     mybir.dt.bfloat16) for ci in range(4)]
        # transposes must land at a PSUM bank base: rotate four bank-sized
        # slots, each transpose writes cols 0:128 of its slot
        NPT = 4
        psT = [pt_(f"psT{i}", [128, 1024], mybir.dt.bfloat16)
               for i in range(NPT)]
        psO = [pt_(f"psO{i}", [128, D_PROJ], mybir.dt.float32)
               for i in range(NPS)]

        isem = sem("isem")    # idx load
        idsem = sem("idsem")  # identity load
        psem = [sem(f"psem{i}") for i in range(7)]   # proj tile loads
        gsem = [sem(f"gsem{i}") for i in range(NB_MAX)]  # per-block gathers
        tsem = sem("tsem")    # PE transposes
        csem = sem("csem")    # DVE chunk copies
        mmsem = sem("mmsem")  # matmul halves
        vcsem = sem("vcsem")  # DVE out casts (h0)
        acsem = sem("acsem")  # ACT out casts (h1)
        osem = sem("osem")    # stores

        # per-core proj DMA schedule: list of (cluster, chunk_lo, n_chunks)
        # c0 is split into 4 DMAs of 2 chunks; others one DMA each.
        def proj_dmas(pjset):
            sched = []
            for ci in sorted(pjset):
                nch = max(CLUSTERS[ci][2] // 128, 1)
                if ci == 0:
                    for c0 in range(0, nch, 2):
                        sched.append((ci, c0, 2))
                else:
                    sched.append((ci, 0, nch))
            return sched

        def section_sync(sy, plan):
            for i, (ci, c0, w) in enumerate(proj_dmas(plan["pj"])):
                part = min(CLUSTERS[ci][2], 128)
                sy.dma_start(
                    pjt_sb[ci][:part, c0 * D_PROJ:(c0 + w) * D_PROJ],
                    pjts[ci][:part, c0 * D_PROJ:(c0 + w) * D_PROJ],
                ).then_inc(psem[i], 16)
            # stores
            nvc = nac = 0
            for b, blk in enumerate(plan["blocks"]):
                nvc += 1
                nac += 1
                sy.wait_ge(vcsem, nvc)
                sy.wait_ge(acsem, nac)
                sy.dma_start(
                    outD[b * 128: b * 128 + blk["mm"], :],
                    OG[:blk["mm"], b * D_PROJ:(b + 1) * D_PROJ],
                ).then_inc(osem, 16)
            sy.wait_ge(osem, 16 * len(plan["blocks"]))

        def section_pool(gp, plan):
            gp.wait_ge(isem, 16)
            for b, blk in enumerate(plan["blocks"]):
                ci = blk["ci"]
                d = CLUSTERS[ci][2]
                gp.indirect_dma_start(
                    G[:, blk["goff"]: blk["goff"] + d], None,
                    tabs[ci][:, :],
                    bass.IndirectOffsetOnAxis(ap=idxt[:, b:b + 1], axis=0),
                ).then_inc(gsem[b], 16)

        def section_pe(te, plan):
            blocks = plan["blocks"]
            pj_sched = proj_dmas(plan["pj"])
            # dma index (0-based) needed for cluster ci chunk c
            def pj_need(ci, c):
                for i, (cj, c0, w) in enumerate(pj_sched):
                    if cj == ci and c0 <= c < c0 + w:
                        return i
                raise AssertionError

            te.wait_ge(idsem, 16)

            cum_copies = [0] * (len(blocks) + 1)
            for b, blk in enumerate(blocks):
                d = CLUSTERS[blk["ci"]][2]
                cum_copies[b + 1] = cum_copies[b] + max(d // 128, 1)

            def emit_T_chunks(b, c_lo, c_hi):
                blk = blocks[b]
                d = CLUSTERS[blk["ci"]][2]
                if c_lo == 0:
                    te.wait_ge(gsem[b], 16)
                for c in range(c_lo, c_hi):
                    w = min(128, d - c * 128)
                    t = cum_copies[b] + c
                    if t >= NPT:
                        te.wait_ge(csem, t - (NPT - 1))
                    te.transpose(
                        psT[t % NPT][:w, 0:128],
                        G[:, blk["goff"] + c * 128: blk["goff"] + c * 128 + w],
                        idn[:, :],
                    ).then_inc(tsem, 1)

            def emit_MM_half(b, h):
                """One accumulation group (half h of block b)."""
                blk = blocks[b]
                ci = blk["ci"]
                d = CLUSTERS[ci][2]
                nch = max(d // 128, 1)
                kd = min(d, 128)
                mm = blk["mm"]
                if h == 0 and b >= NPS:
                    # psO reuse: casts of block b-NPS must be done
                    te.wait_ge(vcsem, b - NPS + 1)
                    te.wait_ge(acsem, b - NPS + 1)
                seen = plan.setdefault("_pj_seen", set())
                last = None
                for c in range(nch):
                    if h == 0:
                        te.wait_ge(csem, cum_copies[b] + c + 1)
                    i_pj = pj_need(ci, c)
                    if (ci, i_pj) not in seen:
                        te.wait_ge(psem[i_pj], 16)
                        seen.add((ci, i_pj))
                    last = te.matmul(
                        psO[b % NPS][:mm, h * 512:(h + 1) * 512],
                        ET[b % NPS][:kd, c * 128: c * 128 + mm],
                        pjt_sb[ci][:kd, c * D_PROJ + h * 512:
                                   c * D_PROJ + h * 512 + 512],
                        start=(c == 0),
                        stop=(c == nch - 1),
                    )
                last.then_inc(mmsem, 1)

            # software pipeline: T(0) up front; then per block b the two
            # matmul groups with the NEXT block's transposes emitted at the
            # group boundaries (PSUM groups never interleave).
            nb = len(blocks)
            emit_T_chunks(0, 0, cum_copies[1] - cum_copies[0])
            for b in range(nb):
                nch_next = (cum_copies[b + 2] - cum_copies[b + 1]
                            if b + 1 < nb else 0)
                emit_MM_half(b, 0)
                if nch_next:
                    emit_T_chunks(b + 1, 0, (nch_next + 1) // 2)
                emit_MM_half(b, 1)
                if nch_next:
                    emit_T_chunks(b + 1, (nch_next + 1) // 2, nch_next)

        def section_dve(ve, plan):
            blocks = plan["blocks"]
            NPT = 4
            nt = 0

            def copy_chunks(b):
                nonlocal nt
                blk = blocks[b]
                d = CLUSTERS[blk["ci"]][2]
                nch = max(d // 128, 1)
                for c in range(nch):
                    w = min(128, d - c * 128)
                    ve.wait_ge(tsem, nt + 1)
                    ve.tensor_copy(
                        ET[b % NPS][:w, c * 128:(c + 1) * 128],
                        psT[nt % NPT][:w, 0:128],
                    ).then_inc(csem, 1)
                    nt += 1

            copy_chunks(0)
            for b, blk in enumerate(blocks):
                mm = blk["mm"]
                ve.wait_ge(mmsem, 2 * b + 1)
                ve.tensor_copy(
                    OG[:mm, b * D_PROJ: b * D_PROJ + 512],
                    psO[b % NPS][:mm, 0:512],
                ).then_inc(vcsem, 1)
                if b + 1 < len(blocks):
                    copy_chunks(b + 1)

        def section_act(sc, plan):
            blocks = plan["blocks"]
            for b, blk in enumerate(blocks):
                mm = blk["mm"]
                sc.wait_ge(mmsem, 2 * (b + 1))
                sc.copy(
                    OG[:mm, b * D_PROJ + 512: b * D_PROJ + 1024],
                    psO[b % NPS][:mm, 512:1024],
                ).then_inc(acsem, 1)

        def dispatch(eng, emit):
            if NO_DISPATCH:
                emit(eng, plans[0])
                return
            pid = eng.partition_id()
            with eng.If_lt(pid, 4):
                with eng.If_lt(pid, 2):
                    with eng.If_eq(pid, 0):
                        emit(eng, plans[0])
                    with eng.Else():
                        emit(eng, plans[1])
                with eng.Else():
                    with eng.If_eq(pid, 2):
                        emit(eng, plans[2])
                    with eng.Else():
                        emit(eng, plans[3])
            with eng.Else():
                with eng.If_lt(pid, 6):
                    with eng.If_eq(pid, 4):
                        emit(eng, plans[4])
                    with eng.Else():
                        emit(eng, plans[5])
                with eng.Else():
                    with eng.If_eq(pid, 6):
                        emit(eng, plans[6])
                    with eng.Else():
                        emit(eng, plans[7])

        def _(sy):
            # idx + identity loads are identical on every core: issue them
            # before the dispatch tree so they are not delayed by the
            # partition-id load
            sy.dma_start(idxt[:, :], idxd[:, :]).then_inc(isem, 16)
            sy.dma_start(idn[:, :], iden[:, :]).then_inc(idsem, 16)
            dispatch(sy, section_sync)
        _(nc.sync)

        def _(gp):
            dispatch(gp, section_pool)
        _(nc.gpsimd)

        def _(te):
            def emit(eng, plan):
                plan.pop("_pj_seen", None)
                section_pe(eng, plan)
            dispatch(te, emit)
        _(nc.tensor)

        def _(ve):
            dispatch(ve, section_dve)
        _(nc.vector)

        def _(sc):
            dispatch(sc, section_act)
        _(nc.scalar)

        nc.compile()
    return nc


def kernel(input, emb0, emb1, emb2, emb3, proj0, proj1, proj2, proj3):
    global LAST_RESULT
    inp = np.asarray(input)
    flat = inp.reshape(-1).astype(np.int64)
    T = flat.shape[0]
    tables = [np.asarray(emb0), np.asarray(emb1), np.asarray(emb2),
              np.asarray(emb3)]
    projs = [np.asarray(proj0), np.asarray(proj1), np.asarray(proj2),
             np.asarray(proj3)]

    rt = _route(flat)
    cores = _balance(rt)

    # --- build per-core plans ---------------------------------------------
    plans = []
    for k in range(N_CORES):
        c = cores[k]
        blocks = []
        goff = 0
        tab_lo = {}  # cluster -> (lo_loc, hi_loc)
        for (ci, s, e) in c["blocks"]:
            loc = rt[ci]["loc"][s:e]
            lo, hi = tab_lo.get(ci, (1 << 60, -1))
            tab_lo[ci] = (min(lo, int(loc.min())), max(hi, int(loc.max())))
        plan = {"pj": c["pj"], "blocks": [], "tab_rows": [0] * 4,
                "tab_base": {}}
        for ci, (lo, hi) in tab_lo.items():
            plan["tab_base"][ci] = lo
            plan["tab_rows"][ci] = hi - lo + 1
        for (ci, s, e) in c["blocks"]:
            d = CLUSTERS[ci][2]
            plan["blocks"].append({
                "ci": ci, "s": s, "e": e, "mm": e - s, "goff": goff,
            })
            goff += d
        assert goff <= G_COLS, f"core {k}: G overflow {goff}"
        assert len(plan["blocks"]) <= NB_MAX
        plans.append(plan)

    nc = _build(plans)

    # --- stage host data ---------------------------------------------------
    tab_rows_max = [max(max((p["tab_rows"][ci] for p in plans)), 1)
                    for ci in range(4)]
    pjt_stage = []
    for ci in range(4):
        d = CLUSTERS[ci][2]
        pt = projs[ci].T.astype(np.float32) * EMB_SCALE  # [d, D_PROJ]
        if d >= 128:
            nch = d // 128
            pt = pt.reshape(nch, 128, D_PROJ).transpose(1, 0, 2)
            pt = pt.reshape(128, nch * D_PROJ)
        pjt_stage.append(np.ascontiguousarray(pt.astype(BF16)))
    iden_np = np.eye(128, dtype=np.float32).astype(BF16)

    in_maps = []
    for k in range(N_CORES):
        plan = plans[k]
        mm = {"iden": iden_np}
        for ci in range(4):
            rows = tab_rows_max[ci]
            d = CLUSTERS[ci][2]
            arr = np.zeros((rows, d), dtype=BF16)
            if plan["tab_rows"][ci] > 0:
                base = plan["tab_base"][ci]
                n = plan["tab_rows"][ci]
                arr[:n] = tables[ci][base: base + n].astype(BF16)
            mm[f"tab{ci}"] = arr
            mm[f"pjt{ci}"] = pjt_stage[ci]
        idx = np.zeros((128, NB_MAX), dtype=np.int32)
        for b, blk in enumerate(plan["blocks"]):
            ci = blk["ci"]
            loc = rt[ci]["loc"][blk["s"]: blk["e"]] - plan["tab_base"][ci]
            idx[: blk["mm"], b] = loc.astype(np.int32)
        mm["idxd"] = idx
        in_maps.append(mm)

    res = run_bass_kernel_spmd(nc, in_maps, core_ids=list(range(N_CORES)))
    LAST_RESULT = res

    # --- unpermute ---------------------------------------------------------
    out_full = np.zeros((T, D_PROJ), np.float32)
    for k in range(N_CORES):
        rows = res.results[k]["out"]
        for b, blk in enumerate(plans[k]["blocks"]):
            ci = blk["ci"]
            pos = rt[ci]["pos"][blk["s"]: blk["e"]]
            out_full[pos] = rows[b * 128: b * 128 + blk["mm"]].astype(
                np.float32)
    return out_full.reshape(*inp.shape, D_PROJ)


# revision 25
# speedup vs baseline: 1.0908x; 1.0460x over previous
"""Adaptive embedding (nn_AdaptiveEmbedding) Trainium2 Bass kernel, v2.

Design: one SPMD program with per-core specialized sections dispatched via a
partition_id() If-tree.  Host routes tokens to cores so each core serves a
small set of clusters (cluster-specialized sharding) -- this removes the 8x
replication of the projection matrices that dominated HBM traffic in v1.

Per 128-token block, the device:
  Pool : one indirect_dma_start (HW dynamic-offset DMA, no ucode library)
         gathering 128 table rows -> SBUF [tokens, d] (token per partition)
  PE   : transposes the gathered tile chunk-wise to [d, tokens] (via identity
         matmul into bf16 PSUM), then runs the projection matmuls into f32
         PSUM, software-pipelined one block ahead of the transposes
  DVE  : copies transposed chunks PSUM->SBUF and casts output half 0
  ACT  : casts output half 1
  SP   : loads idx/identity/projection tiles, stores finished blocks

Host scatters per-core block outputs back to original token positions.
"""

import math
from contextlib import ExitStack

import numpy as np
import ml_dtypes

import concourse.bacc as bacc
import concourse.bass as bass
import concourse.mybir as mybir
from concourse.bass_utils import run_bass_kernel_spmd

N_CORES = 8
D_PROJ = 1024
EMB_SCALE = float(D_PROJ) ** 0.5
BF16 = ml_dtypes.bfloat16

# clusters: (token_left, token_right, d)
CLUSTERS = [
    (0, 20000, 1024),
    (20000, 60000, 256),
    (60000, 100000, 64),
    (100000, 128000, 16),
]

# cost model for the balancer (ns)
POOL_PER_BLOCK = 1410.0
PE_NS_PER_COL = 0.55         # p-state mix
DMA_NS_PER_BYTE = 1.0 / 300.0e9 * 1e9   # 300 GB/s
POOL_START, PE_START, DMA_START = 10500.0, 14500.0, 8300.0
VEC_START = 14500.0
# max number of cores each cluster's blocks (and proj copy) may spread to
SPREAD_CAP = [4, 3, 4, 6]


def _dve_ns(d):
    return max(d // 128, 1) * 300.0 + 530.0  # T-copies + h0 cast

NB_MAX = 16          # max blocks per core the program supports
G_COLS = 8192        # gather buffer cols (bf16) per partition
NPS = 2              # psum double-buffer depth (out tiles and T tiles)
NO_DISPATCH = False  # debug: emit plans[0] for every core, no branching

LAST_RESULT = None


def _pe_cols(d):
    nch = max(d // 128, 1)
    kd = min(d, 128)
    return nch * 128 + nch * 2 * 512  # transposes + matmuls (free-dim cols)


def _block_bytes(d):
    return 128 * d * 2 + 128 * D_PROJ * 2  # gather + out (bf16)


PJ_BYTES = [2 * 1024 * 1024, 512 * 1024, 128 * 1024, 32 * 1024]


def _route(flat):
    """Token routing per cluster: sorted positions and local indices."""
    out = []
    for (l, r, d) in CLUSTERS:
        sel = (flat >= l) & (flat < r)
        pos = np.nonzero(sel)[0]
        loc = (flat[pos] - l).astype(np.int64)
        order = np.argsort(loc, kind="stable")
        out.append({"pos": pos[order], "loc": loc[order], "d": d, "n": len(pos)})
    return out


def _balance(rt):
    """Block assignment minimizing max per-core makespan, with a hard cap on
    how many cores each cluster (and its proj copy) may spread to.

    Returns cores: list of 8 dicts with
      blocks: list of (cluster, start, end) token ranges (<=128 each)
      pj: set of cluster ids present
    Token ranges index into the cluster's sorted token arrays, so each
    core gets a contiguous slice of the sorted-by-loc token list (compact
    vocab slice per core).
    """
    cores = [{"blocks": [], "pj": set(), "pool": 0.0, "pe": 0.0, "dma": 0.0,
              "dve": 0.0}
             for _ in range(N_CORES)]

    def span(c, dpool=0.0, dpe=0.0, ddma=0.0, pj_extra=0, ddve=0.0):
        pjb = sum(PJ_BYTES[p] for p in c["pj"]) + pj_extra
        return max(POOL_START + c["pool"] + dpool,
                   PE_START + c["pe"] + dpe,
                   VEC_START + c["dve"] + ddve,
                   DMA_START + (c["dma"] + ddma + pjb) * DMA_NS_PER_BYTE)

    for ci in [0, 1, 2, 3]:
        n = rt[ci]["n"]
        nblk = (n + 127) // 128
        d = rt[ci]["d"]
        pe_b = _pe_cols(d) * PE_NS_PER_COL
        by_b = _block_bytes(d)
        dve_b = _dve_ns(d)
        counts = [0] * N_CORES
        for _ in range(nblk):
            # candidate cores: those already serving ci, or (if spread cap
            # not hit) any core
            have = [k for k in range(N_CORES) if ci in cores[k]["pj"]]
            cands = list(range(N_CORES)) if len(have) < SPREAD_CAP[ci] else have
            best, bestv = None, None
            for k in cands:
                c = cores[k]
                extra_pj = 0 if ci in c["pj"] else PJ_BYTES[ci]
                v = span(c, POOL_PER_BLOCK, pe_b, by_b, extra_pj, dve_b)
                # tie-break: prefer cores that already carry this proj
                v += (0 if ci in c["pj"] else 1.0)
                if bestv is None or v < bestv - 1e-9:
                    bestv, best = v, k
            c = cores[best]
            c["pool"] += POOL_PER_BLOCK
            c["pe"] += pe_b
            c["dma"] += by_b
            c["dve"] += dve_b
            c["pj"].add(ci)
            counts[best] += 1
        start = 0
        for k in range(N_CORES):
            for _ in range(counts[k]):
                end = min(start + 128, n)
                cores[k]["blocks"].append((ci, start, end))
                start = end
        assert start == n
    return cores


def _build(plans):
    """plans[k]: list of block descriptors:
       (cluster, d, tab_row_offset_base, nblk_index, mm) plus idx data handled
       by host.  We need per-core: blocks list with (cluster, mm)."""
    nc = bacc.Bacc("TRN2", target_bir_lowering=False, num_devices=N_CORES)

    # table shapes: max rows over cores per cluster (host pads)
    tab_rows = [max((p["tab_rows"][ci] for p in plans), default=1) or 1
                for ci in range(4)]
    tabs = [nc.dram_tensor(f"tab{ci}", [max(tab_rows[ci], 1), CLUSTERS[ci][2]],
                           mybir.dt.bfloat16, kind="ExternalInput")
            for ci in range(4)]
    idxd = nc.dram_tensor("idxd", [128, NB_MAX], mybir.dt.int32,
                          kind="ExternalInput")
    iden = nc.dram_tensor("iden", [128, 128], mybir.dt.bfloat16,
                          kind="ExternalInput")
    pjts = [nc.dram_tensor(f"pjt{ci}", [min(CLUSTERS[ci][2], 128),
                                        max(CLUSTERS[ci][2] // 128, 1) * D_PROJ],
                           mybir.dt.bfloat16, kind="ExternalInput")
            for ci in range(4)]
    outD = nc.dram_tensor("out", [NB_MAX * 128, D_PROJ], mybir.dt.bfloat16,
                          kind="ExternalOutput")

    stack = ExitStack()
    sb = lambda name, shape, dt: stack.enter_context(
        nc.sbuf_tensor(name, list(shape), dt))
    pt_ = lambda name, shape, dt: stack.enter_context(
        nc.psum_tensor(name, list(shape), dt))
    sem = lambda name: stack.enter_context(nc.semaphore(name))

    with stack:
        idxt = sb("idxt", [128, NB_MAX], mybir.dt.int32)
        idn = sb("idn", [128, 128], mybir.dt.bfloat16)
        G = sb("G", [128, G_COLS], mybir.dt.bfloat16)
        ET = [sb(f"ET{i}", [128, 1024], mybir.dt.bfloat16) for i in range(NPS)]
        OG = sb("OG", [128, NB_MAX * D_PROJ], mybir.dt.bfloat16)
        pjt_sb = [sb(f"pj{ci}", [min(CLUSTERS[ci][2], 128),
                                 max(CLUSTERS[ci][2] // 128, 1) * D_PROJ],
                     mybir.dt.bfloat16) for ci in range(4)]
        # transposes must land at a PSUM bank base: rotate four bank-sized
        # slots, each transpose writes cols 0:128 of its slot
        NPT = 4
        psT = [pt_(f"psT{i}", [128, 1024], mybir.dt.bfloat16)
               for i in range(NPT)]
        psO = [pt_(f"psO{i}", [128, D_PROJ], mybir.dt.float32)
               for i in range(NPS)]

        isem = sem("isem")    # idx load
        idsem = sem("idsem")  # identity load
        psem = [sem(f"psem{i}") for i in range(7)]   # proj tile loads
        gsem = [sem(f"gsem{i}") for i in range(NB_MAX)]  # per-block gathers
        tsem = sem("tsem")    # PE transposes
        csem = sem("csem")    # DVE chunk copies
        mmsem = sem("mmsem")  # matmul halves
        vcsem = sem("vcsem")  # DVE out casts (h0)
        acsem = sem("acsem")  # ACT out casts (h1)
        osem = sem("osem")    # stores

        # per-core proj DMA schedule: list of (cluster, chunk_lo, n_chunks)
        # c0 is split into 4 DMAs of 2 chunks; others one DMA each.
        def proj_dmas(pjset):
            sched = []
            for ci in sorted(pjset):
                nch = max(CLUSTERS[ci][2] // 128, 1)
                if ci == 0:
                    for c0 in range(0, nch, 2):
                        sched.append((ci, c0, 2))
                else:
                    sched.append((ci, 0, nch))
            return sched

        def section_sync(sy, plan):
            for i, (ci, c0, w) in enumerate(proj_dmas(plan["pj"])):
                part = min(CLUSTERS[ci][2], 128)
                sy.dma_start(
                    pjt_sb[ci][:part, c0 * D_PROJ:(c0 + w) * D_PROJ],
                    pjts[ci][:part, c0 * D_PROJ:(c0 + w) * D_PROJ],
                ).then_inc(psem[i], 16)
            # stores
            nvc = nac = 0
            for b, blk in enumerate(plan["blocks"]):
                nvc += 1
                nac += 1
                sy.wait_ge(vcsem, nvc)
                sy.wait_ge(acsem, nac)
                sy.dma_start(
                    outD[b * 128: b * 128 + blk["mm"], :],
                    OG[:blk["mm"], b * D_PROJ:(b + 1) * D_PROJ],
                ).then_inc(osem, 16)
            sy.wait_ge(osem, 16 * len(plan["blocks"]))

        def section_pool(gp, plan):
            gp.wait_ge(isem, 16)
            for b, blk in enumerate(plan["blocks"]):
                ci = blk["ci"]
                d = CLUSTERS[ci][2]
                gp.indirect_dma_start(
                    G[:, blk["goff"]: blk["goff"] + d], None,
                    tabs[ci][:, :],
                    bass.IndirectOffsetOnAxis(ap=idxt[:, b:b + 1], axis=0),
                ).then_inc(gsem[b], 16)

        def section_pe(te, plan):
            blocks = plan["blocks"]
            pj_sched = proj_dmas(plan["pj"])
            # dma index (0-based) needed for cluster ci chunk c
            def pj_need(ci, c):
                for i, (cj, c0, w) in enumerate(pj_sched):
                    if cj == ci and c0 <= c < c0 + w:
                        return i
                raise AssertionError

            te.wait_ge(idsem, 16)

            cum_copies = [0] * (len(blocks) + 1)
            for b, blk in enumerate(blocks):
                d = CLUSTERS[blk["ci"]][2]
                cum_copies[b + 1] = cum_copies[b] + max(d // 128, 1)

            def emit_T_chunks(b, c_lo, c_hi):
                blk = blocks[b]
                d = CLUSTERS[blk["ci"]][2]
                if c_lo == 0:
                    te.wait_ge(gsem[b], 16)
                for c in range(c_lo, c_hi):
                    w = min(128, d - c * 128)
                    t = cum_copies[b] + c
                    if t >= NPT:
                        te.wait_ge(csem, t - (NPT - 1))
                    te.transpose(
                        psT[t % NPT][:w, 0:128],
                        G[:, blk["goff"] + c * 128: blk["goff"] + c * 128 + w],
                        idn[:, :],
                    ).then_inc(tsem, 1)

            def emit_MM_half(b, h):
                """One accumulation group (half h of block b)."""
                blk = blocks[b]
                ci = blk["ci"]
                d = CLUSTERS[ci][2]
                nch = max(d // 128, 1)
                kd = min(d, 128)
                mm = blk["mm"]
                if h == 0 and b >= NPS:
                    # psO reuse: casts of block b-NPS must be done
                    te.wait_ge(vcsem, b - NPS + 1)
                    te.wait_ge(acsem, b - NPS + 1)
                seen = plan.setdefault("_pj_seen", set())
                last = None
                for c in range(nch):
                    if h == 0:
                        te.wait_ge(csem, cum_copies[b] + c + 1)
                    i_pj = pj_need(ci, c)
                    if (ci, i_pj) not in seen:
                        te.wait_ge(psem[i_pj], 16)
                        seen.add((ci, i_pj))
                    last = te.matmul(
                        psO[b % NPS][:mm, h * 512:(h + 1) * 512],
                        ET[b % NPS][:kd, c * 128: c * 128 + mm],
                        pjt_sb[ci][:kd, c * D_PROJ + h * 512:
                                   c * D_PROJ + h * 512 + 512],
                        start=(c == 0),
                        stop=(c == nch - 1),
                    )
                last.then_inc(mmsem, 1)

            # software pipeline: T(0) up front; then per block b the two
            # matmul groups with the NEXT block's transposes emitted at the
            # group boundaries (PSUM groups never interleave).
            nb = len(blocks)
            emit_T_chunks(0, 0, cum_copies[1] - cum_copies[0])
            for b in range(nb):
                nch_next = (cum_copies[b + 2] - cum_copies[b + 1]
                            if b + 1 < nb else 0)
                emit_MM_half(b, 0)
                if nch_next:
                    emit_T_chunks(b + 1, 0, (nch_next + 1) // 2)
                emit_MM_half(b, 1)
                if nch_next:
                    emit_T_chunks(b + 1, (nch_next + 1) // 2, nch_next)

        def section_dve(ve, plan):
            blocks = plan["blocks"]
            NPT = 4
            nt = 0

            def copy_chunks(b):
                nonlocal nt
                blk = blocks[b]
                d = CLUSTERS[blk["ci"]][2]
                nch = max(d // 128, 1)
                for c in range(nch):
                    w = min(128, d - c * 128)
                    ve.wait_ge(tsem, nt + 1)
                    ve.tensor_copy(
                        ET[b % NPS][:w, c * 128:(c + 1) * 128],
                        psT[nt % NPT][:w, 0:128],
                    ).then_inc(csem, 1)
                    nt += 1

            # copies first, then the cast of the previous block: keeps the
            # per-block PE<->DVE chain shorter than the gather cadence
            copy_chunks(0)
            for b, blk in enumerate(blocks):
                mm = blk["mm"]
                if b + 1 < len(blocks):
                    copy_chunks(b + 1)
                ve.wait_ge(mmsem, 2 * b + 1)
                ve.tensor_copy(
                    OG[:mm, b * D_PROJ: b * D_PROJ + 512],
                    psO[b % NPS][:mm, 0:512],
                ).then_inc(vcsem, 1)

        def section_act(sc, plan):
            blocks = plan["blocks"]
            for b, blk in enumerate(blocks):
                mm = blk["mm"]
                sc.wait_ge(mmsem, 2 * (b + 1))
                sc.copy(
                    OG[:mm, b * D_PROJ + 512: b * D_PROJ + 1024],
                    psO[b % NPS][:mm, 512:1024],
                ).then_inc(acsem, 1)

        def dispatch(eng, emit):
            if NO_DISPATCH:
                emit(eng, plans[0])
                return
            pid = eng.partition_id()
            with eng.If_lt(pid, 4):
                with eng.If_lt(pid, 2):
                    with eng.If_eq(pid, 0):
                        emit(eng, plans[0])
                    with eng.Else():
                        emit(eng, plans[1])
                with eng.Else():
                    with eng.If_eq(pid, 2):
                        emit(eng, plans[2])
                    with eng.Else():
                        emit(eng, plans[3])
            with eng.Else():
                with eng.If_lt(pid, 6):
                    with eng.If_eq(pid, 4):
                        emit(eng, plans[4])
                    with eng.Else():
                        emit(eng, plans[5])
                with eng.Else():
                    with eng.If_eq(pid, 6):
                        emit(eng, plans[6])
                    with eng.Else():
                        emit(eng, plans[7])

        def _(sy):
            # idx + identity loads are identical on every core: issue them
            # before the dispatch tree so they are not delayed by the
            # partition-id load
            sy.dma_start(idxt[:, :], idxd[:, :]).then_inc(isem, 16)
            sy.dma_start(idn[:, :], iden[:, :]).then_inc(idsem, 16)
            dispatch(sy, section_sync)
        _(nc.sync)

        def _(gp):
            dispatch(gp, section_pool)
        _(nc.gpsimd)

        def _(te):
            def emit(eng, plan):
                plan.pop("_pj_seen", None)
                section_pe(eng, plan)
            dispatch(te, emit)
        _(nc.tensor)

        def _(ve):
            dispatch(ve, section_dve)
        _(nc.vector)

        def _(sc):
            dispatch(sc, section_act)
        _(nc.scalar)

        nc.compile()
    return nc


def kernel(input, emb0, emb1, emb2, emb3, proj0, proj1, proj2, proj3):
    global LAST_RESULT
    inp = np.asarray(input)
    flat = inp.reshape(-1).astype(np.int64)
    T = flat.shape[0]
    tables = [np.asarray(emb0), np.asarray(emb1), np.asarray(emb2),
              np.asarray(emb3)]
    projs = [np.asarray(proj0), np.asarray(proj1), np.asarray(proj2),
             np.asarray(proj3)]

    rt = _route(flat)
    cores = _balance(rt)

    # --- build per-core plans ---------------------------------------------
    plans = []
    for k in range(N_CORES):
        c = cores[k]
        blocks = []
        goff = 0
        tab_lo = {}  # cluster -> (lo_loc, hi_loc)
        for (ci, s, e) in c["blocks"]:
            loc = rt[ci]["loc"][s:e]
            lo, hi = tab_lo.get(ci, (1 << 60, -1))
            tab_lo[ci] = (min(lo, int(loc.min())), max(hi, int(loc.max())))
        plan = {"pj": c["pj"], "blocks": [], "tab_rows": [0] * 4,
                "tab_base": {}}
        for ci, (lo, hi) in tab_lo.items():
            plan["tab_base"][ci] = lo
            plan["tab_rows"][ci] = hi - lo + 1
        for (ci, s, e) in c["blocks"]:
            d = CLUSTERS[ci][2]
            plan["blocks"].append({
                "ci": ci, "s": s, "e": e, "mm": e - s, "goff": goff,
            })
            goff += d
        assert goff <= G_COLS, f"core {k}: G overflow {goff}"
        assert len(plan["blocks"]) <= NB_MAX
        plans.append(plan)

    nc = _build(plans)

    # --- stage host data ---------------------------------------------------
    tab_rows_max = [max(max((p["tab_rows"][ci] for p in plans)), 1)
                    for ci in range(4)]
    pjt_stage = []
    for ci in range(4):
        d = CLUSTERS[ci][2]
        pt = projs[ci].T.astype(np.float32) * EMB_SCALE  # [d, D_PROJ]
        if d >= 128:
            nch = d // 128
            pt = pt.reshape(nch, 128, D_PROJ).transpose(1, 0, 2)
            pt = pt.reshape(128, nch * D_PROJ)
        pjt_stage.append(np.ascontiguousarray(pt.astype(BF16)))
    iden_np = np.eye(128, dtype=np.float32).astype(BF16)

    in_maps = []
    for k in range(N_CORES):
        plan = plans[k]
        mm = {"iden": iden_np}
        for ci in range(4):
            rows = tab_rows_max[ci]
            d = CLUSTERS[ci][2]
            arr = np.zeros((rows, d), dtype=BF16)
            if plan["tab_rows"][ci] > 0:
                base = plan["tab_base"][ci]
                n = plan["tab_rows"][ci]
                arr[:n] = tables[ci][base: base + n].astype(BF16)
            mm[f"tab{ci}"] = arr
            mm[f"pjt{ci}"] = pjt_stage[ci]
        idx = np.zeros((128, NB_MAX), dtype=np.int32)
        for b, blk in enumerate(plan["blocks"]):
            ci = blk["ci"]
            loc = rt[ci]["loc"][blk["s"]: blk["e"]] - plan["tab_base"][ci]
            idx[: blk["mm"], b] = loc.astype(np.int32)
        mm["idxd"] = idx
        in_maps.append(mm)

    res = run_bass_kernel_spmd(nc, in_maps, core_ids=list(range(N_CORES)))
    LAST_RESULT = res

    # --- unpermute ---------------------------------------------------------
    out_full = np.zeros((T, D_PROJ), np.float32)
    for k in range(N_CORES):
        rows = res.results[k]["out"]
        for b, blk in enumerate(plans[k]["blocks"]):
            ci = blk["ci"]
            pos = rt[ci]["pos"][blk["s"]: blk["e"]]
            out_full[pos] = rows[b * 128: b * 128 + blk["mm"]].astype(
                np.float32)
    return out_full.reshape(*inp.shape, D_PROJ)
